# revision 50
# baseline (speedup 1.0000x reference)
"""Trainium2 Bass kernel for nn_DeformRouting (deformable routing conv).

Sharding: 8 cores, data-parallel over N x H-halves: core c handles image
n = c//2, row-half = c%2 (14 rows x 28 cols = 392 pixels).

Math (per pixel pt, output channel o; m = c*9+kk):
  out[o, pt] = x[o,pt] * sum_m w_wgt[o,m] * samp[m,pt]
             + sum_m b_wgt.reshape(64,576)[o,m] * samp[m,pt]
where samp[(c,kk), pt] is the bilinear sample of x[c] at the deformed
position of tap kk for pixel pt.

Device pipeline per core (points-on-partitions, 4 chunks of 98 pts):
  1. offset conv: 4 PE matmuls -> ps_off[pt, ch, 18] (PSUM, read in place).
  2. index math (short chain, all coords pre-shifted positive so
     floor == i32-trunc): ix = off*13.5 + base' -> floor -> clip ->
     idx = 29*ycl + xcl (fp16, exact ints).
  3. idx wrap to the gather's 16-partition layout: 8 PE perm-matmuls
     (fp16) + one tensor_scalar(-928) PSUM->i16 copy.
  4. per-chunk SWDGE dma_gather (1152 idx x 1KB) from a host-built
     29x29 2x2-PATCH table (fp16): one descriptor fetches all four
     bilinear taps [A0|A1|B0|B1] x 64ch. Chunk gathers pipeline: chunk
     c's DMA + combine run under chunk c+1's descriptor generation.
  5. bilinear weights (fp16, off critical path) + combine (7 DVE ops
     per chunk, fp16).
  6. per-chunk PE transposes (fp16) -> rhs[(c,kk), pt] and 10
     accumulating fp16 matmuls -> ps1/ps2; out = ps1*x + ps2 (f32).
"""

import numpy as np

import concourse.bass as bass
import concourse.tile as tile
from concourse import bacc, mybir
from concourse.bass_utils import run_bass_kernel_spmd

# problem constants (hardcoded per contract)
N, CIN, COUT, H, W, K = 4, 64, 64, 28, 28, 3
K2 = K * K  # 9
NCORES = 8
HHALF = H // 2          # 14 rows per core
NPT = HHALF * W         # 392 points per core
PCH = 98                # points per partition-chunk
NCH = 4                 # chunks (4*98 = 392)
TBL_S = H + 1           # 29 y-floor slots
TBL_T = W + 1           # 29 x-floor slots
TBL_ROWS = TBL_S * TBL_T  # 841 patch rows
SC = (W - 1) / 2.0      # 13.5
XOFF = 30.0             # x floor-shift: xf = round(ix + 30 - 0.5) = floor(ix) + 30
YOFF = 32.0             # y floor-shift: yf = round(iy + 32 - 0.5) = floor(iy) + 32
# The DVE f32->i32 cast rounds to nearest, so the host bakes (XOFF - 0.5)
# into the base grid and the fractional weight adds the 0.5 back.
# idx = 29*(ycl-31) + (xcl-29) = 29*ycl + xcl - 928
IDX_BIAS = 928.0
NI = K2 * 128           # 1152 gather descriptors per chunk
NB = 5                  # ceil(576/128) contraction chunks

F32 = mybir.dt.float32
F16 = mybir.dt.float16
I32 = mybir.dt.int32
I16 = mybir.dt.int16

_CACHE = {}


def _alu(name):
    return getattr(mybir.AluOpType, name)


def _build_program():
    nc = bacc.Bacc("TRN2", target_bir_lowering=False, debug=False,
                   num_devices=NCORES)

    # DRAM I/O (per-core shapes)
    # packed f16 input: [xc16(512) | wofft(18) | ident(128) | wwb(640) |
    #                    mg(1024)] = 2322 f16 per partition
    tbl = nc.dram_tensor("tbl", [TBL_ROWS, 4 * CIN], F16, kind="ExternalInput")
    pf16a = nc.dram_tensor("pf16a", [128, 530], F16, kind="ExternalInput")
    pf16b = nc.dram_tensor("pf16b", [128, 1792], F16, kind="ExternalInput")
    pf32 = nc.dram_tensor("pf32", [128, 2 * NCH * K2], F32,
                          kind="ExternalInput")
    xcf = nc.dram_tensor("xcf", [COUT, NPT], F32, kind="ExternalInput")
    out_d = nc.dram_tensor("out", [COUT, NPT], F32, kind="ExternalOutput")

    mult, add, sub = _alu("mult"), _alu("add"), _alu("subtract")
    is_eq = _alu("is_equal")
    amin, amax = _alu("min"), _alu("max")

    with tile.TileContext(nc) as tc:
        with (
            tc.tile_pool(name="const", bufs=1) as cpool,
            tc.tile_pool(name="work", bufs=1) as wpool,
            tc.tile_pool(name="psoff", bufs=1, space="PSUM") as opool,
            tc.tile_pool(name="pst", bufs=2, space="PSUM") as ppool,
            tc.tile_pool(name="pso", bufs=2, space="PSUM") as popool,
        ):
            # ---- packed input loads (conv deps land first) ----
            biga = cpool.tile([128, 530], F16)
            nc.sync.dma_start(biga[:], pf16a.ap())
            big32 = cpool.tile([128, NCH, 2 * K2], F32)
            nc.sync.dma_start(big32[:], pf32.ap().rearrange(
                "p (a b) -> p a b", a=NCH))
            bigb = cpool.tile([128, 1792], F16)
            nc.sync.dma_start(bigb[:], pf16b.ap())
            xcf_sb = cpool.tile([COUT, NPT], F32)
            nc.sync.dma_start(xcf_sb[:], xcf.ap())

            xc_sb = biga[:, 0:512]
            wofft_v = biga[:, 512:530]
            id_v = bigb[:, 0:128]
            wwb_v = bigb[:, 128:768].rearrange("p (a b) -> p a b", a=10)
            mg_v = bigb[:, 768:1792].rearrange("p (a b) -> p a b", a=8)
            baseb_v = big32[:]

            # warmup gather: absorbs the Q7 gather-ucode launch latency
            # before the real gathers (no data deps beyond a memset)
            widx = wpool.tile([128, 1], I16, name="widx")
            nc.vector.memset(widx[:], 0)
            wga = wpool.tile([128, 1, 4 * CIN], F16, name="wga")
            nc.gpsimd.dma_gather(
                out_ap=wga[:], in_ap=tbl.ap(), idxs_ap=widx[:],
                num_idxs=16, num_idxs_reg=16, elem_size=4 * CIN,
                single_packet=False)

            # ---- per-chunk: conv -> index math -> wrap -> gather.
            # Per-chunk chains let chunk 0's gather start ~4us earlier than
            # a whole-tensor pipeline; later chunks' math runs under the
            # earlier chunks' descriptor generation.
            KA = 6
            ps_off = opool.tile([128, NCH, 2 * K2], F32)
            ibs, w_tiles = [], []
            gas = [None] * (NCH - 1)
            ga3a = ga3b = None
            for ch in range(NCH):
                nc.tensor.matmul(
                    out=ps_off[:, ch, :],
                    lhsT=xc_sb[:, ch * PCH:ch * PCH + 128],
                    rhs=wofft_v,
                    start=True, stop=True,
                )
                # index math: f32->i32 cast rounds to nearest; with the
                # -0.5 host bake that IS the floor; clip commutes with the
                # rounding cast for integer bounds. x/y interleaved
                # ([128, 18], x even / y odd).
                ib = wpool.tile([128, 2 * K2], F32, name=f"ib{ch}")
                nc.vector.scalar_tensor_tensor(
                    ib[:], ps_off[:, ch, :], SC, baseb_v[:, ch, :],
                    mult, add)
                cl_i = wpool.tile([128, 2 * K2], I32, name=f"cl{ch}")
                nc.vector.tensor_scalar(cl_i[:, 0::2], ib[:, 0::2],
                                        57.0, 29.0, amin, amax)
                nc.vector.tensor_scalar(cl_i[:, 1::2], ib[:, 1::2],
                                        59.0, 31.0, amin, amax)
                idxh = wpool.tile([128, K2], F16, name=f"idxh{ch}")
                nc.vector.scalar_tensor_tensor(idxh[:], cl_i[:, 1::2],
                                               float(TBL_T), cl_i[:, 0::2],
                                               mult, add)
                ibs.append((ib, cl_i))

                psw = opool.tile([128, 8, K2], F32, tag="psw",
                                 name=f"psw{ch}")
                for gsel in range(8):
                    nc.tensor.matmul(
                        out=psw[:, gsel, :], lhsT=mg_v[:, gsel, :],
                        rhs=idxh[:], start=True, stop=True)
                wrap = wpool.tile([128, K2, 8], I16, name=f"wrap{ch}")
                nc.vector.tensor_scalar(
                    wrap[:].rearrange("q m g -> q g m"), psw[:],
                    IDX_BIAS, None, sub)

                def gather(name, mlo, nk):
                    ga = wpool.tile([128, nk, 4 * CIN], F16, name=name)
                    nc.gpsimd.dma_gather(
                        out_ap=ga[:],
                        in_ap=tbl.ap(),
                        idxs_ap=wrap[:, mlo:mlo + nk, :].rearrange(
                            "q m g -> q (m g)"),
                        num_idxs=nk * 128, num_idxs_reg=nk * 128,
                        elem_size=4 * CIN, single_packet=False)
                    return ga

                if ch < NCH - 1:
                    gas[ch] = gather(f"ga{ch}", 0, K2)
                else:
                    ga3a = gather("ga3a", 0, KA)
                    ga3b = gather("ga3b", KA, K2 - KA)

            # ---- bilinear weights (fp16), consumed only by the combine.
            # No validity ops: out-of-range taps hit zero-sentinel halves
            # of the patch table (t/s == 0 or 28) and the frac weights are
            # clamped to [0,1], collapsing far-out-of-range weights to 0.
            for ch in range(NCH):
                ib, cl_i = ibs[ch]
                cl_f = wpool.tile([128, 2 * K2], F32, name=f"clf{ch}")
                nc.vector.tensor_copy(cl_f[:], cl_i[:])
                w1b = wpool.tile([128, 2 * K2], F32, name=f"w1b{ch}")
                nc.vector.scalar_tensor_tensor(w1b[:], ib[:], 0.5, cl_f[:],
                                               add, sub)
                nc.vector.tensor_scalar(w1b[:], w1b[:], 1.0, 0.0, amin, amax)
                w0b = wpool.tile([128, 2 * K2], F32, name=f"w0b{ch}")
                nc.vector.scalar_tensor_tensor(w0b[:], cl_f[:], 0.5, ib[:],
                                               add, sub)
                nc.vector.tensor_scalar(w0b[:], w0b[:], 1.0, 0.0, amin, amax)

                def tap(wy, wx, name):
                    o = wpool.tile([128, K2], F16, name=name)
                    nc.vector.tensor_tensor(o[:], wy[:, 1::2], wx[:, 0::2],
                                            mult)
                    return o

                w_tiles.append((tap(w0b, w0b, f"w00_{ch}"),
                                tap(w1b, w0b, f"w10_{ch}"),
                                tap(w0b, w1b, f"w01_{ch}"),
                                tap(w1b, w1b, f"w11_{ch}")))

            def combine(samp, ga, ch, k0, nk, tag):
                # tree-structured: 4 independent mults, then 3 adds
                # (depth 3, issues back-to-back on the DVE)
                sv = samp[:]
                w00c, w10c, w01c, w11c = w_tiles[ch]

                def bc(wt):
                    return wt[:, k0:k0 + nk, None].to_broadcast(
                        [128, nk, CIN])

                ts = [wpool.tile([128, nk, CIN], F16, name=f"ct{tag}{j}")
                      for j in range(3)]
                nc.vector.tensor_tensor(sv, ga[:, :, 0:CIN], bc(w00c), mult)
                nc.vector.tensor_tensor(ts[0][:], ga[:, :, CIN:2 * CIN],
                                        bc(w10c), mult)
                nc.vector.tensor_tensor(ts[1][:], ga[:, :, 2 * CIN:3 * CIN],
                                        bc(w01c), mult)
                nc.vector.tensor_tensor(ts[2][:], ga[:, :, 3 * CIN:],
                                        bc(w11c), mult)
                nc.vector.tensor_tensor(sv, sv, ts[0][:], add)
                nc.vector.tensor_tensor(ts[1][:], ts[1][:], ts[2][:], add)
                nc.vector.tensor_tensor(sv, sv, ts[1][:], add)

            def transposes(rhs, samp, blo, bhi, b0):
                sflat = samp[:].rearrange("p k c -> p (k c)")
                for b in range(blo, bhi):
                    mlo = 128 * (b - b0)
                    mhi = min(mlo + 128, samp.shape[1] * CIN)
                    pstile = ppool.tile([128, 128], F16, tag="tps")
                    nc.tensor.transpose(
                        pstile[:mhi - mlo, :], sflat[:, mlo:mhi], id_v)
                    nc.scalar.copy(
                        rhs[:mhi - mlo, b, :], pstile[:mhi - mlo, :PCH])

            def finals(rhs, ps1c, ps2c, blo, bhi):
                for ps, boff in ((ps1c, 0), (ps2c, NB)):
                    for b in range(blo, bhi):
                        nc.tensor.matmul(
                            out=ps[:], lhsT=wwb_v[:, boff + b, :],
                            rhs=rhs[:, b, :],
                            start=(b == 0), stop=(b == NB - 1))

            def emit_out(ch, ps1c, ps2c):
                cols = slice(ch * PCH, (ch + 1) * PCH)
                osb = wpool.tile([COUT, PCH], F32, name=f"osb{ch}")
                nc.vector.tensor_tensor(osb[:], ps1c[:], xcf_sb[:, cols],
                                        mult)
                nc.vector.tensor_tensor(osb[:], osb[:], ps2c[:], add)
                nc.sync.dma_start(out_d.ap()[:, cols], osb[:])

            for ch in range(NCH - 1):
                samp = wpool.tile([128, K2, CIN], F16, name=f"samp{ch}")
                combine(samp, gas[ch], ch, 0, K2, f"c{ch}")
                rhs = wpool.tile([128, NB, PCH], F16, name=f"rhs{ch}")
                transposes(rhs, samp, 0, NB, 0)
                nc.vector.memset(rhs[CIN * K2 - 512:, NB - 1, :], 0.0)
                ps1c = popool.tile([COUT, PCH], F32, tag="ps1",
                                   name=f"ps1_{ch}")
                ps2c = popool.tile([COUT, PCH], F32, tag="ps2",
                                   name=f"ps2_{ch}")
                finals(rhs, ps1c, ps2c, 0, NB)
                emit_out(ch, ps1c, ps2c)

            # last chunk: halves pipelined in separate tiles (kk 0..3 =
            # m-blocks 0,1; kk 4..8 = m-blocks 2,3,4)
            ch = NCH - 1
            samp3a = wpool.tile([128, KA, CIN], F16, name="samp3a")
            samp3b = wpool.tile([128, K2 - KA, CIN], F16, name="samp3b")
            rhs = wpool.tile([128, NB, PCH], F16, name=f"rhs{ch}")
            combine(samp3a, ga3a, ch, 0, KA, "c3a")
            transposes(rhs, samp3a, 0, 3, 0)
            combine(samp3b, ga3b, ch, KA, K2 - KA, "c3b")
            transposes(rhs, samp3b, 3, NB, 3)
            nc.vector.memset(rhs[CIN * K2 - 512:, NB - 1, :], 0.0)
            ps1c = popool.tile([COUT, PCH], F32, tag="ps1", name="ps1_3")
            ps2c = popool.tile([COUT, PCH], F32, tag="ps2", name="ps2_3")
            finals(rhs, ps1c, ps2c, 0, NB)
            emit_out(ch, ps1c, ps2c)

    nc.compile()
    return nc


def _host_inputs(x, w_off, b_off, w_wgt, b_wgt):
    """Build the 8 per-core input dicts (layout/shard prep only)."""
    x = np.asarray(x, dtype=np.float32)
    w_off = np.asarray(w_off, dtype=np.float32)
    b_off = np.asarray(b_off, dtype=np.float32)
    w_wgt = np.asarray(w_wgt, dtype=np.float32)
    b_wgt = np.asarray(b_wgt, dtype=np.float32)

    xs = np.linspace(-1.0, 1.0, W).astype(np.float32)
    ys = np.linspace(-1.0, 1.0, H).astype(np.float32)
    kx = np.linspace(-(K - 1) / (W - 1), (K - 1) / (W - 1), K).astype(np.float32)
    ky = np.linspace(-(K - 1) / (H - 1), (K - 1) / (H - 1), K).astype(np.float32)

    # wwb [128, 10, 64] fp16, contraction rows m = kk*64 + c (kk-major, to
    # match the device's contiguous samp layout): chunks 0..4 =
    # W~[m, o] = w_wgt[o, c*9+kk] (zero-pad 576->640), chunks 5..9 =
    # B~[m, o] = b_wgt.reshape(64, 576)[o, c*9+kk].
    m_new = np.arange(K2 * CIN)
    m_old = (m_new % CIN) * K2 + (m_new // CIN)   # (kk,c) -> c*9+kk
    wtp = np.zeros((640, COUT), dtype=np.float32)
    wtp[:576] = w_wgt.T[m_old]
    btp = np.zeros((640, COUT), dtype=np.float32)
    btp[:576] = b_wgt.reshape(CIN, K2 * COUT).T[m_old]
    wwb = np.concatenate([wtp.reshape(5, 128, COUT),
                          btp.reshape(5, 128, COUT)], axis=0)
    wwb = wwb.transpose(1, 0, 2).reshape(128, 10 * COUT).astype(np.float16)

    # idx-wrap permutation selectors: mg[pt, g*128+q] = (pt == g*16 + q%16)
    mgm = np.zeros((128, 8, 128), dtype=np.float16)
    q = np.arange(128)
    for gsel in range(8):
        mgm[gsel * 16 + (q % 16), gsel, q] = 1.0
    mgm = mgm.reshape(128, 8 * 128)

    wofft = np.zeros((128, 2 * K2), dtype=np.float16)
    wofft[:CIN] = w_off.T.astype(np.float16)
    ident = np.eye(128, dtype=np.float16)

    # patch-table row/col clip maps
    rt = np.clip(np.arange(TBL_S) - 1, 0, H - 1)
    rb = np.clip(np.arange(TBL_S), 0, H - 1)
    ct = np.clip(np.arange(TBL_T) - 1, 0, W - 1)
    cr = np.clip(np.arange(TBL_T), 0, W - 1)

    in_maps = []
    for c in range(NCORES):
        n, half = divmod(c, 2)
        r0 = HHALF * half
        xn = x[n]                             # [64, 28, 28]
        x_hwc = xn.transpose(1, 2, 0)         # [28, 28, 64]

        # 2x2 patch table [841, 256] fp16: row (s,t) =
        # [x[rt,ct] | x[rb,ct] | x[rt,cr] | x[rb,cr]] with zero sentinels
        # where a tap is out of range (replaces on-device validity math)
        tbl = np.concatenate([
            x_hwc[rt][:, ct], x_hwc[rb][:, ct],
            x_hwc[rt][:, cr], x_hwc[rb][:, cr],
        ], axis=-1).astype(np.float16)        # [29, 29, 256]
        tbl[:, 0, 0:128] = 0       # t=0: x0 = -1 -> A0, A1 zero
        tbl[:, TBL_T - 1, 128:256] = 0  # t=28: x1 = 28 -> B0, B1 zero
        tbl[0, :, 0:64] = 0        # s=0: y0 = -1 -> A0 zero
        tbl[0, :, 128:192] = 0     # s=0: B0 zero
        tbl[TBL_S - 1, :, 64:128] = 0   # s=28: y1 = 28 -> A1 zero
        tbl[TBL_S - 1, :, 192:256] = 0  # s=28: B1 zero

        xslice = xn.reshape(CIN, H * W)[:, r0 * W:r0 * W + NPT]
        xcpad = np.zeros((128, 512), dtype=np.float16)
        xcpad[:CIN, :NPT] = xslice.astype(np.float16)

        # base grids [128, NCH, K2] with the floor-shift bakes (-0.5 turns
        # the round-to-nearest cast into a floor)
        bx = np.full((128, NCH, K2), SC + XOFF - 0.5, dtype=np.float32)
        by = np.full((128, NCH, K2), SC + YOFF - 0.5, dtype=np.float32)
        p_idx = np.arange(PCH)
        for ch in range(NCH):
            g = r0 * W + ch * PCH + p_idx          # global pixel
            row, col = g // W, g % W
            for kk in range(K2):
                kyi, kxi = divmod(kk, K)
                bx[:PCH, ch, kk] = (xs[col] + kx[kxi] + b_off[2 * kk]
                                    + 1.0) * SC + XOFF - 0.5
                by[:PCH, ch, kk] = (ys[row] + ky[kyi] + b_off[2 * kk + 1]
                                    + 1.0) * SC + YOFF - 0.5

        pf16a = np.concatenate([xcpad, wofft], axis=1)      # [128, 530]
        pf16b = np.concatenate([ident, wwb, mgm], axis=1)   # [128, 1792]
        # interleave x/y bases: [128, NCH, 18] with x at even, y at odd
        bb = np.empty((128, NCH, 2 * K2), dtype=np.float32)
        bb[:, :, 0::2] = bx
        bb[:, :, 1::2] = by
        pf32 = bb.reshape(128, 2 * NCH * K2)
        in_maps.append({
            "tbl": tbl.reshape(TBL_ROWS, 4 * CIN),
            "pf16a": np.ascontiguousarray(pf16a),
            "pf16b": np.ascontiguousarray(pf16b),
            "pf32": np.ascontiguousarray(pf32),
            "xcf": np.ascontiguousarray(xslice[:COUT]),
        })
    return in_maps


def get_program():
    if "nc" not in _CACHE:
        _CACHE["nc"] = _build_program()
    return _CACHE["nc"]


def run_cores(in_maps, **kw):
    nc = get_program()
    return run_bass_kernel_spmd(nc, in_maps, core_ids=list(range(NCORES)), **kw)


def assemble(results):
    out = np.zeros((N, COUT, H, W), dtype=np.float32)
    for c in range(NCORES):
        n, half = divmod(c, 2)
        out[n, :, HHALF * half:HHALF * (half + 1), :] = \
            results[c]["out"].reshape(COUT, HHALF, W)
    return out


def kernel(x, w_off, b_off, w_wgt, b_wgt):
    in_maps = _host_inputs(x, w_off, b_off, w_wgt, b_wgt)
    res = run_cores(in_maps)
    return assemble(res.results)


# revision 51
# speedup vs baseline: 1.1596x; 1.1596x over previous
"""Trainium2 Bass kernel for nn_DeformRouting (deformable routing conv).

Sharding: 8 cores, data-parallel over N x H-halves: core c handles image
n = c//2, row-half = c%2 (14 rows x 28 cols = 392 pixels).

Math (per pixel pt, output channel o; m = c*9+kk):
  out[o, pt] = x[o,pt] * sum_m w_wgt[o,m] * samp[m,pt]
             + sum_m b_wgt.reshape(64,576)[o,m] * samp[m,pt]
where samp[(c,kk), pt] is the bilinear sample of x[c] at the deformed
position of tap kk for pixel pt.

Device pipeline per core (points-on-partitions, 4 chunks of 98 pts):
  1. offset conv: 4 PE matmuls -> ps_off[pt, ch, 18] (PSUM, read in place).
  2. index math (short chain, all coords pre-shifted positive so
     floor == i32-trunc): ix = off*13.5 + base' -> floor -> clip ->
     idx = 29*ycl + xcl (fp16, exact ints).
  3. idx wrap to the gather's 16-partition layout: 8 PE perm-matmuls
     (fp16) + one tensor_scalar(-928) PSUM->i16 copy.
  4. per-chunk SWDGE dma_gather (1152 idx x 1KB) from a host-built
     29x29 2x2-PATCH table (fp16): one descriptor fetches all four
     bilinear taps [A0|A1|B0|B1] x 64ch. Chunk gathers pipeline: chunk
     c's DMA + combine run under chunk c+1's descriptor generation.
  5. bilinear weights (fp16, off critical path) + combine (7 DVE ops
     per chunk, fp16).
  6. per-chunk PE transposes (fp16) -> rhs[(c,kk), pt] and 10
     accumulating fp16 matmuls -> ps1/ps2; out = ps1*x + ps2 (f32).
"""

import numpy as np

import concourse.bass as bass
import concourse.tile as tile
from concourse import bacc, mybir
from concourse.bass_utils import run_bass_kernel_spmd

# problem constants (hardcoded per contract)
N, CIN, COUT, H, W, K = 4, 64, 64, 28, 28, 3
K2 = K * K  # 9
NCORES = 8
HHALF = H // 2          # 14 rows per core
NPT = HHALF * W         # 392 points per core
PCH = 98                # points per partition-chunk
NCH = 4                 # chunks (4*98 = 392)
TBL_S = H + 1           # 29 y-floor slots
TBL_T = W + 1           # 29 x-floor slots
TBL_ROWS = TBL_S * TBL_T  # 841 patch rows
SC = (W - 1) / 2.0      # 13.5
XOFF = 30.0             # x floor-shift: xf = round(ix + 30 - 0.5) = floor(ix) + 30
YOFF = 32.0             # y floor-shift: yf = round(iy + 32 - 0.5) = floor(iy) + 32
# The DVE f32->i32 cast rounds to nearest, so the host bakes (XOFF - 0.5)
# into the base grid and the fractional weight adds the 0.5 back.
# idx = 29*(ycl-31) + (xcl-29) = 29*ycl + xcl - 928
IDX_BIAS = 928.0
NI = K2 * 128           # 1152 gather descriptors per chunk
NB = 5                  # ceil(576/128) contraction chunks

F32 = mybir.dt.float32
F16 = mybir.dt.float16
I32 = mybir.dt.int32
I16 = mybir.dt.int16

_CACHE = {}


def _alu(name):
    return getattr(mybir.AluOpType, name)


def _build_program():
    nc = bacc.Bacc("TRN2", target_bir_lowering=False, debug=False,
                   num_devices=NCORES)

    # DRAM I/O (per-core shapes)
    # packed f16 input: [xc16(512) | wofft(18) | ident(128) | wwb(640) |
    #                    mg(1024)] = 2322 f16 per partition
    tbl = nc.dram_tensor("tbl", [TBL_ROWS, 4 * CIN], F16, kind="ExternalInput")
    pf16a = nc.dram_tensor("pf16a", [128, 530], F16, kind="ExternalInput")
    pf16b = nc.dram_tensor("pf16b", [128, 1792], F16, kind="ExternalInput")
    pf32 = nc.dram_tensor("pf32", [128, 2 * NCH * K2], F32,
                          kind="ExternalInput")
    xcf = nc.dram_tensor("xcf", [COUT, NPT], F32, kind="ExternalInput")
    out_d = nc.dram_tensor("out", [COUT, NPT], F32, kind="ExternalOutput")

    mult, add, sub = _alu("mult"), _alu("add"), _alu("subtract")
    is_eq = _alu("is_equal")
    amin, amax = _alu("min"), _alu("max")

    with tile.TileContext(nc) as tc:
        with (
            tc.tile_pool(name="const", bufs=1) as cpool,
            tc.tile_pool(name="work", bufs=1) as wpool,
            tc.tile_pool(name="psoff", bufs=1, space="PSUM") as opool,
            tc.tile_pool(name="pst", bufs=2, space="PSUM") as ppool,
            tc.tile_pool(name="pso", bufs=2, space="PSUM") as popool,
        ):
            # ---- packed input loads (conv deps land first) ----
            biga = cpool.tile([128, 530], F16)
            nc.sync.dma_start(biga[:], pf16a.ap())
            big32 = cpool.tile([128, NCH, 2 * K2], F32)
            nc.sync.dma_start(big32[:], pf32.ap().rearrange(
                "p (a b) -> p a b", a=NCH))
            bigb = cpool.tile([128, 1792], F16)
            nc.sync.dma_start(bigb[:], pf16b.ap())
            xcf_sb = cpool.tile([COUT, NPT], F32)
            nc.sync.dma_start(xcf_sb[:], xcf.ap())

            xc_sb = biga[:, 0:512]
            wofft_v = biga[:, 512:530]
            id_v = bigb[:, 0:128]
            wwb_v = bigb[:, 128:768].rearrange("p (a b) -> p a b", a=10)
            mg_v = bigb[:, 768:1792].rearrange("p (a b) -> p a b", a=8)
            baseb_v = big32[:]

            # ---- 1. offset conv: ps_off[pt, ch, 18] ----
            KA = 6
            ps_off = opool.tile([128, NCH, 2 * K2], F32)
            for ch in range(NCH):
                nc.tensor.matmul(
                    out=ps_off[:, ch, :],
                    lhsT=xc_sb[:, ch * PCH:ch * PCH + 128],
                    rhs=wofft_v,
                    start=True, stop=True,
                )

            # ---- 2. index math. The f32->i32 cast rounds to nearest; with
            # the -0.5 host bake that IS the floor; clip commutes with the
            # rounding cast for integer bounds. x/y interleaved
            # ([128, NCH, 18], x even / y odd).
            shp2 = [128, NCH, 2 * K2]
            ib = wpool.tile(shp2, F32, name="ib")
            nc.vector.scalar_tensor_tensor(ib[:], ps_off[:], SC, baseb_v,
                                           mult, add)
            cl_i = wpool.tile(shp2, I32, name="cl_i")
            nc.vector.tensor_scalar(cl_i[:, :, 0::2], ib[:, :, 0::2],
                                    57.0, 29.0, amin, amax)
            nc.vector.tensor_scalar(cl_i[:, :, 1::2], ib[:, :, 1::2],
                                    59.0, 31.0, amin, amax)
            idxh = wpool.tile([128, NCH, K2], F16, name="idxh")
            nc.vector.scalar_tensor_tensor(idxh[:], cl_i[:, :, 1::2],
                                           float(TBL_T), cl_i[:, :, 0::2],
                                           mult, add)

            # ---- 3. wrap idx into the gather's 16-partition layout ----
            psw = opool.tile([128, 8, NCH * K2], F32, name="psw")
            idxv = idxh[:].rearrange("p a b -> p (a b)")
            for gsel in range(8):
                nc.tensor.matmul(
                    out=psw[:, gsel, :], lhsT=mg_v[:, gsel, :], rhs=idxv,
                    start=True, stop=True)
            # one wrap tile per chunk (tile-granularity dep tracking would
            # otherwise gate chunk 0's gather on all four)
            wraps = []
            for ch in range(NCH):
                w = wpool.tile([128, K2, 8], I16, name=f"wrap{ch}")
                nc.vector.tensor_scalar(
                    w[:].rearrange("q m g -> q g m"),
                    psw[:, :, ch * K2:(ch + 1) * K2], IDX_BIAS, None, sub)
                wraps.append(w)

            # ---- 4. per-chunk gathers (emitted before the weight math so
            # their DVE gate covers only the wrap ops); last chunk split
            # kk 0..5 / 6..8 to shorten the tail ----
            def gather(name, ch, mlo, nk):
                ga = wpool.tile([128, nk, 4 * CIN], F16, name=name)
                nc.gpsimd.dma_gather(
                    out_ap=ga[:],
                    in_ap=tbl.ap(),
                    idxs_ap=wraps[ch][:, mlo:mlo + nk, :].rearrange(
                        "q m g -> q (m g)"),
                    num_idxs=nk * 128, num_idxs_reg=nk * 128,
                    elem_size=4 * CIN, single_packet=False)
                return ga

            gas = [gather(f"ga{ch}", ch, 0, K2) for ch in range(NCH - 1)]
            ga3a = gather("ga3a", NCH - 1, 0, KA)
            ga3b = gather("ga3b", NCH - 1, KA, K2 - KA)

            # ---- 5. bilinear weights (fp16), combine-only consumers.
            # No validity ops: out-of-range taps hit zero-sentinel halves
            # of the patch table and frac weights are clamped to [0,1].
            cl_f = wpool.tile(shp2, F32, name="cl_f")
            nc.vector.tensor_copy(cl_f[:], cl_i[:])
            w1b = wpool.tile(shp2, F32, name="w1b")
            nc.vector.scalar_tensor_tensor(w1b[:], ib[:], 0.5, cl_f[:],
                                           add, sub)
            nc.vector.tensor_scalar(w1b[:], w1b[:], 1.0, 0.0, amin, amax)
            w0b = wpool.tile(shp2, F32, name="w0b")
            nc.vector.scalar_tensor_tensor(w0b[:], cl_f[:], 0.5, ib[:],
                                           add, sub)
            nc.vector.tensor_scalar(w0b[:], w0b[:], 1.0, 0.0, amin, amax)

            w_tiles = []
            for ch in range(NCH):
                def tap(wy, wx, name):
                    o = wpool.tile([128, K2], F16, name=name)
                    nc.vector.tensor_tensor(o[:], wy[:, ch, 1::2],
                                            wx[:, ch, 0::2], mult)
                    return o

                w_tiles.append((tap(w0b, w0b, f"w00_{ch}"),
                                tap(w1b, w0b, f"w10_{ch}"),
                                tap(w0b, w1b, f"w01_{ch}"),
                                tap(w1b, w1b, f"w11_{ch}")))

            def combine(samp, ga, ch, k0, nk, tag):
                # tree-structured: 4 independent mults, then 3 adds
                # (depth 3, issues back-to-back on the DVE)
                sv = samp[:]
                w00c, w10c, w01c, w11c = w_tiles[ch]

                def bc(wt):
                    return wt[:, k0:k0 + nk, None].to_broadcast(
                        [128, nk, CIN])

                ts = [wpool.tile([128, nk, CIN], F16, name=f"ct{tag}{j}")
                      for j in range(3)]
                nc.vector.tensor_tensor(sv, ga[:, :, 0:CIN], bc(w00c), mult)
                nc.vector.tensor_tensor(ts[0][:], ga[:, :, CIN:2 * CIN],
                                        bc(w10c), mult)
                nc.vector.tensor_tensor(ts[1][:], ga[:, :, 2 * CIN:3 * CIN],
                                        bc(w01c), mult)
                nc.vector.tensor_tensor(ts[2][:], ga[:, :, 3 * CIN:],
                                        bc(w11c), mult)
                nc.vector.tensor_tensor(sv, sv, ts[0][:], add)
                nc.vector.tensor_tensor(ts[1][:], ts[1][:], ts[2][:], add)
                nc.vector.tensor_tensor(sv, sv, ts[1][:], add)

            def transposes(rhs, samp, blo, bhi, b0):
                sflat = samp[:].rearrange("p k c -> p (k c)")
                for b in range(blo, bhi):
                    mlo = 128 * (b - b0)
                    mhi = min(mlo + 128, samp.shape[1] * CIN)
                    pstile = ppool.tile([128, 128], F16, tag="tps")
                    nc.tensor.transpose(
                        pstile[:mhi - mlo, :], sflat[:, mlo:mhi], id_v)
                    nc.scalar.copy(
                        rhs[:mhi - mlo, b, :], pstile[:mhi - mlo, :PCH])

            def finals(rhs, ps1c, ps2c, blo, bhi):
                for ps, boff in ((ps1c, 0), (ps2c, NB)):
                    for b in range(blo, bhi):
                        nc.tensor.matmul(
                            out=ps[:], lhsT=wwb_v[:, boff + b, :],
                            rhs=rhs[:, b, :],
                            start=(b == 0), stop=(b == NB - 1))

            def emit_out(ch, ps1c, ps2c):
                cols = slice(ch * PCH, (ch + 1) * PCH)
                osb = wpool.tile([COUT, PCH], F32, name=f"osb{ch}")
                nc.vector.tensor_tensor(osb[:], ps1c[:], xcf_sb[:, cols],
                                        mult)
                nc.vector.tensor_tensor(osb[:], osb[:], ps2c[:], add)
                nc.sync.dma_start(out_d.ap()[:, cols], osb[:])

            for ch in range(NCH - 1):
                samp = wpool.tile([128, K2, CIN], F16, name=f"samp{ch}")
                combine(samp, gas[ch], ch, 0, K2, f"c{ch}")
                rhs = wpool.tile([128, NB, PCH], F16, name=f"rhs{ch}")
                transposes(rhs, samp, 0, NB, 0)
                nc.vector.memset(rhs[CIN * K2 - 512:, NB - 1, :], 0.0)
                ps1c = popool.tile([COUT, PCH], F32, tag="ps1",
                                   name=f"ps1_{ch}")
                ps2c = popool.tile([COUT, PCH], F32, tag="ps2",
                                   name=f"ps2_{ch}")
                finals(rhs, ps1c, ps2c, 0, NB)
                emit_out(ch, ps1c, ps2c)

            # last chunk: halves pipelined in separate tiles (kk 0..3 =
            # m-blocks 0,1; kk 4..8 = m-blocks 2,3,4)
            ch = NCH - 1
            samp3a = wpool.tile([128, KA, CIN], F16, name="samp3a")
            samp3b = wpool.tile([128, K2 - KA, CIN], F16, name="samp3b")
            rhs = wpool.tile([128, NB, PCH], F16, name=f"rhs{ch}")
            combine(samp3a, ga3a, ch, 0, KA, "c3a")
            transposes(rhs, samp3a, 0, 3, 0)
            combine(samp3b, ga3b, ch, KA, K2 - KA, "c3b")
            transposes(rhs, samp3b, 3, NB, 3)
            nc.vector.memset(rhs[CIN * K2 - 512:, NB - 1, :], 0.0)
            ps1c = popool.tile([COUT, PCH], F32, tag="ps1", name="ps1_3")
            ps2c = popool.tile([COUT, PCH], F32, tag="ps2", name="ps2_3")
            finals(rhs, ps1c, ps2c, 0, NB)
            emit_out(ch, ps1c, ps2c)

    nc.compile()
    return nc


def _host_inputs(x, w_off, b_off, w_wgt, b_wgt):
    """Build the 8 per-core input dicts (layout/shard prep only)."""
    x = np.asarray(x, dtype=np.float32)
    w_off = np.asarray(w_off, dtype=np.float32)
    b_off = np.asarray(b_off, dtype=np.float32)
    w_wgt = np.asarray(w_wgt, dtype=np.float32)
    b_wgt = np.asarray(b_wgt, dtype=np.float32)

    xs = np.linspace(-1.0, 1.0, W).astype(np.float32)
    ys = np.linspace(-1.0, 1.0, H).astype(np.float32)
    kx = np.linspace(-(K - 1) / (W - 1), (K - 1) / (W - 1), K).astype(np.float32)
    ky = np.linspace(-(K - 1) / (H - 1), (K - 1) / (H - 1), K).astype(np.float32)

    # wwb [128, 10, 64] fp16, contraction rows m = kk*64 + c (kk-major, to
    # match the device's contiguous samp layout): chunks 0..4 =
    # W~[m, o] = w_wgt[o, c*9+kk] (zero-pad 576->640), chunks 5..9 =
    # B~[m, o] = b_wgt.reshape(64, 576)[o, c*9+kk].
    m_new = np.arange(K2 * CIN)
    m_old = (m_new % CIN) * K2 + (m_new // CIN)   # (kk,c) -> c*9+kk
    wtp = np.zeros((640, COUT), dtype=np.float32)
    wtp[:576] = w_wgt.T[m_old]
    btp = np.zeros((640, COUT), dtype=np.float32)
    btp[:576] = b_wgt.reshape(CIN, K2 * COUT).T[m_old]
    wwb = np.concatenate([wtp.reshape(5, 128, COUT),
                          btp.reshape(5, 128, COUT)], axis=0)
    wwb = wwb.transpose(1, 0, 2).reshape(128, 10 * COUT).astype(np.float16)

    # idx-wrap permutation selectors: mg[pt, g*128+q] = (pt == g*16 + q%16)
    mgm = np.zeros((128, 8, 128), dtype=np.float16)
    q = np.arange(128)
    for gsel in range(8):
        mgm[gsel * 16 + (q % 16), gsel, q] = 1.0
    mgm = mgm.reshape(128, 8 * 128)

    wofft = np.zeros((128, 2 * K2), dtype=np.float16)
    wofft[:CIN] = w_off.T.astype(np.float16)
    ident = np.eye(128, dtype=np.float16)

    # patch-table row/col clip maps
    rt = np.clip(np.arange(TBL_S) - 1, 0, H - 1)
    rb = np.clip(np.arange(TBL_S), 0, H - 1)
    ct = np.clip(np.arange(TBL_T) - 1, 0, W - 1)
    cr = np.clip(np.arange(TBL_T), 0, W - 1)

    in_maps = []
    for c in range(NCORES):
        n, half = divmod(c, 2)
        r0 = HHALF * half
        xn = x[n]                             # [64, 28, 28]
        x_hwc = xn.transpose(1, 2, 0)         # [28, 28, 64]

        # 2x2 patch table [841, 256] fp16: row (s,t) =
        # [x[rt,ct] | x[rb,ct] | x[rt,cr] | x[rb,cr]] with zero sentinels
        # where a tap is out of range (replaces on-device validity math)
        tbl = np.concatenate([
            x_hwc[rt][:, ct], x_hwc[rb][:, ct],
            x_hwc[rt][:, cr], x_hwc[rb][:, cr],
        ], axis=-1).astype(np.float16)        # [29, 29, 256]
        tbl[:, 0, 0:128] = 0       # t=0: x0 = -1 -> A0, A1 zero
        tbl[:, TBL_T - 1, 128:256] = 0  # t=28: x1 = 28 -> B0, B1 zero
        tbl[0, :, 0:64] = 0        # s=0: y0 = -1 -> A0 zero
        tbl[0, :, 128:192] = 0     # s=0: B0 zero
        tbl[TBL_S - 1, :, 64:128] = 0   # s=28: y1 = 28 -> A1 zero
        tbl[TBL_S - 1, :, 192:256] = 0  # s=28: B1 zero

        xslice = xn.reshape(CIN, H * W)[:, r0 * W:r0 * W + NPT]
        xcpad = np.zeros((128, 512), dtype=np.float16)
        xcpad[:CIN, :NPT] = xslice.astype(np.float16)

        # base grids [128, NCH, K2] with the floor-shift bakes (-0.5 turns
        # the round-to-nearest cast into a floor)
        bx = np.full((128, NCH, K2), SC + XOFF - 0.5, dtype=np.float32)
        by = np.full((128, NCH, K2), SC + YOFF - 0.5, dtype=np.float32)
        p_idx = np.arange(PCH)
        for ch in range(NCH):
            g = r0 * W + ch * PCH + p_idx          # global pixel
            row, col = g // W, g % W
            for kk in range(K2):
                kyi, kxi = divmod(kk, K)
                bx[:PCH, ch, kk] = (xs[col] + kx[kxi] + b_off[2 * kk]
                                    + 1.0) * SC + XOFF - 0.5
                by[:PCH, ch, kk] = (ys[row] + ky[kyi] + b_off[2 * kk + 1]
                                    + 1.0) * SC + YOFF - 0.5

        pf16a = np.concatenate([xcpad, wofft], axis=1)      # [128, 530]
        pf16b = np.concatenate([ident, wwb, mgm], axis=1)   # [128, 1792]
        # interleave x/y bases: [128, NCH, 18] with x at even, y at odd
        bb = np.empty((128, NCH, 2 * K2), dtype=np.float32)
        bb[:, :, 0::2] = bx
        bb[:, :, 1::2] = by
        pf32 = bb.reshape(128, 2 * NCH * K2)
        in_maps.append({
            "tbl": tbl.reshape(TBL_ROWS, 4 * CIN),
            "pf16a": np.ascontiguousarray(pf16a),
            "pf16b": np.ascontiguousarray(pf16b),
            "pf32": np.ascontiguousarray(pf32),
            "xcf": np.ascontiguousarray(xslice[:COUT]),
        })
    return in_maps


def get_program():
    if "nc" not in _CACHE:
        _CACHE["nc"] = _build_program()
    return _CACHE["nc"]


def run_cores(in_maps, **kw):
    nc = get_program()
    return run_bass_kernel_spmd(nc, in_maps, core_ids=list(range(NCORES)), **kw)


def assemble(results):
    out = np.zeros((N, COUT, H, W), dtype=np.float32)
    for c in range(NCORES):
        n, half = divmod(c, 2)
        out[n, :, HHALF * half:HHALF * (half + 1), :] = \
            results[c]["out"].reshape(COUT, HHALF, W)
    return out


def kernel(x, w_off, b_off, w_wgt, b_wgt):
    in_maps = _host_inputs(x, w_off, b_off, w_wgt, b_wgt)
    res = run_cores(in_maps)
    return assemble(res.results)
